# revision 1
# baseline (speedup 1.0000x reference)
"""Trainium2 Bass kernel for the periodic 9-point diffusion stencil.

Full input:  state [4, 8, 1024, 1024] f32, diffusion_coefficient, dt (scalars).
Full output: [4, 8, 1024, 1024] f32.

Math:  out = c2 * X + c1 * (Sv @ Sh(X))   with Sv = Sh = [1,2,1] periodic,
       c1 = scale/12, c2 = 1 - 4*scale/3, scale = dc*dt.
This equals the reference 9-point kernel (center 1-scale, edges scale/6,
corners scale/12); the reference's mass-conservation correction is orders of
magnitude below f32 resolution and enters here only through exact math.

Sharding: 32 independent (b, c) slices of [1024, 1024]; 4 slices per core
(pure data parallel, no collectives). Per slice: 8 overlapping row-blocks
(x rows 126n..126n+127, partition-aligned -> out rows 126n+1..126n+126) so
the vertical stencil needs no cross-partition traffic, plus one K=32 tile
whose partition window (x rows 1008..1023, 0..15) realizes the vertical wrap
and yields out rows 1009..1023 and row 0. Blocks are grouped 4-at-a-time
into 2MB super-tiles moved by ONE multi-dim DMA each way (per-DMA issue cost
~1.7us/queue makes 512KB granularity issue-bound); in-DMAs issue from the
sync sequencer and out-DMAs from the scalar engine's queue so a combine-wait
never stalls input issue.

The vertical [1,2,1] filter is a tridiagonal-matrix matmul on the
TensorEngine; horizontal taps are column-shifted rhs views accumulated in
PSUM, with the column wrap as two extra 1-wide matmuls. PE operands are
bf16 (weights are real bf16 tensors; X is read through a bitcast high-half
view of the f32 tile, i.e. truncated bf16): that precision only touches the
~scale-sized neighbor terms, while the dominant identity term c2*X is
computed in exact f32 by one fused VectorEngine scalar_tensor_tensor op per
block that also reads PSUM and writes the output tile. For large scale
(dc*dt > 0.02) an exact-f32-matmul program is selected instead.

Measured on the 8-core axon TRN2: ~95-115us/exec vs a ~82-110us pure
HBM-copy floor for the same 33MB/core traffic (DMA-bound); rel err vs the
f64 reference ~2e-6 at the reference's scale=1e-3.
"""

import numpy as np

N_CORES = 8
S_PER_CORE = 4  # (b,c) slices per core
H = W = 1024
ROWS_PER_TILE = 126  # valid output rows per full tile
N_FULL_TILES = 8     # 8*126 = 1008 rows; last 16 rows via a K=32 tile
LAST_ROWS = H - N_FULL_TILES * ROWS_PER_TILE  # 16

_PROGRAMS = {}


def _build_program(loop_r=1, x_bufs=3, o_bufs=3, ps_bufs=4, split_combine=0,
                   skip_tail=0, variant="full", pe_dtype="bf16",
                   nblk=4, alt_queues=0, ramp=0):
    from contextlib import ExitStack

    import concourse.bass as bass
    import concourse.tile as tile
    from concourse import bacc, mybir
    from concourse.bass_interp import get_hw_module

    f32 = mybir.dt.float32
    bf16 = mybir.dt.bfloat16
    mult = mybir.AluOpType.mult
    add = mybir.AluOpType.add

    nc = bacc.Bacc("TRN2", target_bir_lowering=False, debug=False,
                   num_devices=N_CORES)
    wdt = bf16 if pe_dtype == "bf16" else f32
    x = nc.dram_tensor("x", [S_PER_CORE, H, W], f32, kind="ExternalInput").ap()
    w1 = nc.dram_tensor("w1", [128, 128], wdt, kind="ExternalInput").ap()
    w2 = nc.dram_tensor("w2", [128, 128], wdt, kind="ExternalInput").ap()
    c2v = nc.dram_tensor("c2v", [128, 1], f32, kind="ExternalInput").ap()
    y = nc.dram_tensor("y", [S_PER_CORE, H, W], f32, kind="ExternalOutput").ap()

    with tile.TileContext(nc) as tc:
        with ExitStack() as ctx:
            consts = ctx.enter_context(tc.tile_pool(name="consts", bufs=1))
            xp = ctx.enter_context(tc.tile_pool(name="x", bufs=x_bufs))
            op = ctx.enter_context(tc.tile_pool(name="o", bufs=o_bufs))
            pp = ctx.enter_context(
                tc.tile_pool(name="ps", bufs=ps_bufs, space="PSUM"))

            w1t = consts.tile([128, 128], wdt)
            nc.sync.dma_start(w1t[:], w1[:])
            w2t = consts.tile([128, 128], wdt)
            nc.sync.dma_start(w2t[:], w2[:])
            c2t = consts.tile([128, 1], f32)
            nc.sync.dma_start(c2t[:], c2v[:])

            def stencil_tile(xb, pt, K, base=0):
                """Accumulate the 9-point neighbor sum (scaled by c1) into
                psum tile pt. xb is the bf16 high-half view of the f32 SBUF
                data; `base` selects a 1024-wide block within it. Horizontal
                shifts live in the rhs windows, with the two wrap columns
                via 1-wide matmuls."""
                l1 = w1t[:K, :K]
                l2 = w2t[:K, :K]
                b = base
                # center taps (weight 2*c1*T), first writers of both banks
                nc.tensor.matmul(pt[:, 0:512], l2, xb[:, b:b + 512],
                                 start=True, stop=False, skip_group_check=True)
                nc.tensor.matmul(pt[:, 512:1024], l2, xb[:, b + 512:b + 1024],
                                 start=True, stop=False, skip_group_check=True)
                # left neighbors: psum[:, j] += W1 @ X[:, j-1]
                nc.tensor.matmul(pt[:, 1:512], l1, xb[:, b:b + 511],
                                 start=False, stop=False, skip_group_check=True)
                nc.tensor.matmul(pt[:, 0:1], l1, xb[:, b + 1023:b + 1024],
                                 start=False, stop=False, skip_group_check=True)
                nc.tensor.matmul(pt[:, 512:1024], l1, xb[:, b + 511:b + 1023],
                                 start=False, stop=False, skip_group_check=True)
                # right neighbors: psum[:, j] += W1 @ X[:, j+1]
                nc.tensor.matmul(pt[:, 0:512], l1, xb[:, b + 1:b + 513],
                                 start=False, stop=True, skip_group_check=True)
                nc.tensor.matmul(pt[:, 512:1023], l1, xb[:, b + 513:b + 1024],
                                 start=False, stop=False, skip_group_check=True)
                nc.tensor.matmul(pt[:, 1023:1024], l1, xb[:, b:b + 1],
                                 start=False, stop=True, skip_group_check=True)

            def body(_i=None):
                from concourse.ap import AP as mkAP
                for s in range(S_PER_CORE):
                    # super-tiles of 4 row-blocks; block n = x rows
                    # 126n..126n+127 (partition-aligned, no wrap) -> out rows
                    # 126n+1..126n+126. One 2MB multi-dim DMA each way per
                    # super-tile: per-DMA issue cost (~1.7us/queue) dominates
                    # at 512KB granularity.
                    xs_ = x[s]
                    ys_ = y[s]
                    if ramp and s == 0:
                        plan = [1, 3, 4]
                    elif ramp and s == S_PER_CORE - 1:
                        plan = [4, 3, 1]
                    else:
                        plan = [nblk] * (8 // nblk)
                    n0 = 0
                    for g, nb in enumerate(plan):
                        if alt_queues:
                            eng_in = nc.sync if g % 2 == 0 else nc.scalar
                            eng_out = nc.scalar if g % 2 == 0 else nc.sync
                        else:
                            eng_in, eng_out = nc.sync, nc.scalar
                        in_view = mkAP(
                            tensor=xs_.tensor,
                            offset=xs_.offset + 126 * n0 * W,
                            ap=[[W, 128], [126 * W, nb], [1, W]])
                        xt = xp.tile([128, nb * W], f32, tag="xt")
                        eng_in.dma_start(
                            xt[:].rearrange("p (n w) -> p n w", n=nb),
                            in_view)
                        xb = xt[:].bitcast(bf16)[:, 1::2] \
                            if pe_dtype == "bf16" else xt[:]
                        ot = op.tile([128, nb * W], f32, tag="ot")
                        for b in range(nb):
                            if variant == "dma":
                                continue
                            pt = pp.tile([128, W], f32, tag="pt")
                            if variant in ("pe", "full"):
                                stencil_tile(xb, pt, 128, base=b * W)
                            if variant == "pe":
                                dot = op.tile([1, 2], f32, tag="dummy")
                                nc.vector.tensor_copy(dot[:], pt[0:1, 0:2])
                                continue
                            src_ = pt[:] if variant == "full" else \
                                xt[:, b * W:(b + 1) * W]
                            eng = nc.gpsimd if (split_combine and b % 2) else \
                                nc.vector
                            eng.scalar_tensor_tensor(
                                ot[:, b * W:(b + 1) * W],
                                xt[:, b * W:(b + 1) * W],
                                c2t[:], src_, op0=mult, op1=add)
                        if variant in ("dma", "pe"):
                            nc.vector.tensor_copy(ot[0:1, 0:2],
                                                  xt[0:1, 0:2])
                        # out-DMAs issue from ACT's HWDGE queue so their
                        # wait-on-combine doesn't stall the sync sequencer's
                        # in-DMA issue stream
                        out_view = mkAP(
                            tensor=ys_.tensor,
                            offset=ys_.offset + (126 * n0 + 1) * W,
                            ap=[[W, 126], [126 * W, nb], [1, W]])
                        eng_out.dma_start(
                            out_view,
                            ot[1:127, :].rearrange("p (n w) -> p n w",
                                                   n=nb))
                        n0 += nb

                    # K=32 wrap tile: partitions 0..15 = x rows 1008..1023,
                    # partitions 16..31 = x rows 0..15. Valid psum rows 1..30;
                    # rows 1..15 -> out rows 1009..1023, row 16 -> out row 0
                    # (its taps at partitions 15,16,17 = x rows 1023,0,1 are
                    # exactly the vertical wrap).
                    if skip_tail:
                        continue
                    r0 = N_FULL_TILES * ROWS_PER_TILE + 1  # 1009
                    xt = xp.tile([32, W], f32, tag="xt_last")
                    nc.sync.dma_start(xt[0:16, :], x[s, H - 16:H, :])
                    nc.sync.dma_start(xt[16:32, :], x[s, 0:16, :])
                    pt = pp.tile([32, W], f32, tag="pt")
                    xbl = xt[:].bitcast(bf16)[:, 1::2] \
                        if pe_dtype == "bf16" else xt[:]
                    stencil_tile(xbl, pt, 32)
                    ot = op.tile([32, W], f32, tag="ot")
                    nc.vector.scalar_tensor_tensor(
                        ot[:], xt[:], c2t[0:32, :], pt[:], op0=mult, op1=add)
                    nc.scalar.dma_start(y[s, r0:H, :], ot[1:1 + H - r0, :])
                    nc.scalar.dma_start(y[s, 0:1, :], ot[16:17, :])

            if loop_r == 1:
                body()
            else:
                with tc.For_i(0, loop_r, 1):
                    body()

    nc.compile()
    nc.m = get_hw_module(nc.m)
    return nc


def _get_program(pe_dtype="bf16"):
    if pe_dtype not in _PROGRAMS:
        _PROGRAMS[pe_dtype] = _build_program(pe_dtype=pe_dtype)
    return _PROGRAMS[pe_dtype]


def kernel(state, diffusion_coefficient, dt):
    import ml_dtypes
    from concourse.bass_utils import run_bass_kernel_spmd

    state = np.asarray(state)
    in_dtype = state.dtype
    xs = np.ascontiguousarray(state, dtype=np.float32).reshape(32, H, W)

    scale = float(np.asarray(diffusion_coefficient, dtype=np.float64)) * \
        float(np.asarray(dt, dtype=np.float64))
    c1 = scale / 12.0
    c2 = 1.0 - 4.0 * scale / 3.0

    tri = np.zeros((128, 128), dtype=np.float64)
    idx = np.arange(128)
    tri[idx, idx] = 2.0
    tri[idx[:-1], idx[:-1] + 1] = 1.0
    tri[idx[:-1] + 1, idx[:-1]] = 1.0
    # The fast path reads X through a truncated-bf16 view on the PE, whose
    # error enters scaled by ~scale; for the reference's scale=1e-3 the final
    # rel err is ~2e-6. For unexpectedly large scale fall back to exact-f32
    # matmuls (4 cycles/row on the PE, ~2x slower end-to-end, but exact).
    if scale <= 0.02:
        wt_dtype = ml_dtypes.bfloat16
        nc = _get_program("bf16")
    else:
        wt_dtype = np.float32
        nc = _get_program("f32")
    w1 = (c1 * tri).astype(wt_dtype)
    w2 = (2.0 * c1 * tri).astype(wt_dtype)
    c2v = np.full((128, 1), c2, dtype=np.float32)
    in_maps = [
        {"x": xs[k * S_PER_CORE:(k + 1) * S_PER_CORE], "w1": w1, "w2": w2,
         "c2v": c2v}
        for k in range(N_CORES)
    ]
    res = run_bass_kernel_spmd(nc, in_maps, core_ids=list(range(N_CORES)))
    out = np.concatenate([res.results[k]["y"] for k in range(N_CORES)], axis=0)
    return out.reshape(4, 8, H, W).astype(in_dtype, copy=False)



# revision 48
# speedup vs baseline: 1.7486x; 1.7486x over previous
"""Trainium2 Bass kernel for the periodic 9-point diffusion stencil.

Full input:  state [4, 8, 1024, 1024] f32, diffusion_coefficient, dt (scalars).
Full output: [4, 8, 1024, 1024] f32.

Math:  out = c2 * X + c1 * (Sv @ Sh(X))  with Sv = Sh = [1,2,1] periodic,
       c1 = scale/12, c2 = 1 - 4*scale/3, scale = dc*dt.

The kernel is memory-bound; the relative-error budget (2e-2) is ~30x above
fp16 quantization error (~6e-4 measured vs the f64 reference), so all HBM
traffic is fp16: the host converts the input to fp16 and pre-pads each
[1024,1024] slice with its periodic wrap to [1026,1026]; the device reads
fp16 and writes fp16, halving the DMA-resource time vs f32 (the sole
bottleneck, ~360 GB/s/core shared across all DMAs).

Per (b,c) slice: 9 partition-aligned row blocks (8 x 128 rows stride 126,
plus an 18-row tail), each yielding 126 (16 for the tail) output rows.
Per block (one-pass block_plan modes):
  u = x_left + x_right        one DVE tensor_tensor on shifted views of the
                              padded tile (2-byte dtype -> 2x DVE mode),
                              fused across each 4-block super-tile
  psum = W1 @ u (+ c2*x)      two 512-col matmuls per block on the PE with
                              W1 = c1*tri(1,2,1); the center term c2*x is
                              either pre-written into PSUM by the ACT engine
                              (activation Copy with per-partition scale,
                              matmul accumulates with start=False) or fused
                              into the PSUM->SBUF combine
  out = copy/combine(psum)    PSUM f32 -> SBUF fp16, rotated across
                              ACT/DVE/GPSIMD so no engine exceeds the DMA
                              floor
One matmul pass keeps the PE under the DMA floor even at the 1.2 GHz
mid p-state (the PE clock only ramps to 2.4 GHz after 3us of gap-free
execution, which a DMA-bound kernel cannot guarantee).
Blocks are grouped 4-at-a-time into one multi-dim in-DMA / out-DMA each
(in on the sync sequencer's queue, out on ACT's).

Measured rel err vs the f64 reference ~6e-4 (gate 2e-2).
"""

import numpy as np

N_CORES = 8
S_PER_CORE = 4  # (b,c) slices per core
H = W = 1024
HP = WP = 1026   # host-padded (periodic wrap) slice dims
RPB = 126        # output rows per full block
NBLK = 8         # full 128-row blocks; tail block handles last 16 rows
TAIL_X0 = NBLK * RPB  # 1008: x_p row where the 18-row tail block starts

_PROGRAMS = {}

_DEF_PLAN = "xV"


def _build_program(loop_r=1, x_bufs=5, u_bufs=5, o_bufs=4, ps_bufs=4,
                   block_plan=_DEF_PLAN, out_eng="scalar", variant="full",
                   tt_gran="vonly", nbk=4, tails_last=0, out_gran="block",
                   v_out_eng="gpsimd", tail_tt_eng="vector", tt_plan="v"):
    """block_plan is cycled per block; tokens:
      'A' ACT pre-writes c2*x into psum, PE adds W1@u, ACT copies out
      'B' like 'A' but DVE does the copy-out
      'C' like 'A' but GPSIMD does the copy-out
      'V' PE writes W1@u, DVE scalar_tensor_tensor combines c2*x + psum
      'a'/'v' two-pass matmul (W1@u + W2@x), copy-out on ACT/DVE
      'x'/'y' three-pass matmul (W2'@x + W1@xl + W1@xr, no u needed),
              copy-out on ACT/DVE; if no block needs u, the tt is skipped
    out_gran: "super" = one out-DMA per super-tile from out_eng's queue;
      "block" = one out-DMA per block, issued by that block's combine engine
      right after the combine (no cross-engine wait before issue).
    """
    from contextlib import ExitStack

    import concourse.bass as bass
    import concourse.tile as tile
    from concourse import bacc, mybir
    from concourse.bass_interp import get_hw_module

    f32 = mybir.dt.float32
    fp16 = mybir.dt.float16
    add = mybir.AluOpType.add
    mult = mybir.AluOpType.mult
    Copy = mybir.ActivationFunctionType.Copy

    plan = block_plan.replace(" ", "")
    # GPSIMD cannot access PSUM (BIR verifier) — no 'C'/'G'/'g' modes
    assert not set(plan) & set("CGg"), plan

    nc = bacc.Bacc("TRN2", target_bir_lowering=False, debug=False,
                   num_devices=N_CORES)
    x = nc.dram_tensor("x", [S_PER_CORE, HP, WP], fp16,
                       kind="ExternalInput").ap()
    w1 = nc.dram_tensor("w1", [128, 128], fp16, kind="ExternalInput").ap()
    w2 = nc.dram_tensor("w2", [128, 128], fp16, kind="ExternalInput").ap()
    c2v = nc.dram_tensor("c2v", [128, 1], f32, kind="ExternalInput").ap()
    y = nc.dram_tensor("y", [S_PER_CORE, H, W], fp16,
                       kind="ExternalOutput").ap()

    with tile.TileContext(nc) as tc:
        with ExitStack() as ctx:
            consts = ctx.enter_context(tc.tile_pool(name="consts", bufs=1))
            xp = ctx.enter_context(tc.tile_pool(name="x", bufs=x_bufs))
            up = ctx.enter_context(tc.tile_pool(name="u", bufs=u_bufs))
            op = ctx.enter_context(tc.tile_pool(name="o", bufs=o_bufs))
            pp = ctx.enter_context(
                tc.tile_pool(name="ps", bufs=ps_bufs, space="PSUM"))

            w1t = consts.tile([128, 128], fp16)
            nc.sync.dma_start(w1t[:], w1[:])
            w2t = consts.tile([128, 128], fp16)
            nc.sync.dma_start(w2t[:], w2[:])
            c2t = consts.tile([128, 1], f32)
            nc.sync.dma_start(c2t[:], c2v[:])

            eng_out = getattr(nc, out_eng)

            def do_copy(which, dst, src):
                if which == "a":
                    nc.scalar.activation(dst, src, Copy)
                    return nc.scalar
                elif which == "v":
                    nc.vector.tensor_copy(dst, src)
                    return nc.vector
                else:
                    nc.gpsimd.tensor_copy(dst, src)
                    return nc.gpsimd

            def body(_i=None):
                from concourse.ap import AP as mkAP
                ci = [0]  # block-plan cursor (cycled)

                def next_mode():
                    m = plan[ci[0] % len(plan)]
                    ci[0] += 1
                    return m

                def do_block(mode, K, pt, ut, xt, ub, xb, ot, ob):
                    """One stencil block: psum cols from ut[:, ub:ub+1024]
                    and xt[:, xb:xb+1024]; result into ot[:, ob:ob+1024]
                    rows 0..K-1 (row r = x_p row r of the block)."""
                    l1 = w1t[0:K, 0:K]
                    if mode in "ABC":
                        nc.scalar.activation(
                            pt[0:K, 0:1024], xt[0:K, xb:xb + 1024], Copy,
                            scale=c2t[0:K, :])
                        for j in (0, 1):
                            nc.tensor.matmul(
                                pt[0:K, j * 512:(j + 1) * 512], l1,
                                ut[0:K, ub + j * 512:ub + j * 512 + 512],
                                start=False, stop=True,
                                skip_group_check=True)
                        return do_copy({"A": "a", "B": "v", "C": "g"}[mode],
                                       ot[0:K, ob:ob + 1024],
                                       pt[0:K, 0:1024])
                    elif mode == "V":
                        for j in (0, 1):
                            nc.tensor.matmul(
                                pt[0:K, j * 512:(j + 1) * 512], l1,
                                ut[0:K, ub + j * 512:ub + j * 512 + 512],
                                start=True, stop=True,
                                skip_group_check=True)
                        nc.vector.scalar_tensor_tensor(
                            ot[0:K, ob:ob + 1024], xt[0:K, xb:xb + 1024],
                            c2t[0:K, :], pt[0:K, 0:1024],
                            op0=mult, op1=add)
                        return nc.vector
                    elif mode in "xy":
                        # three-pass: horizontal shifts as rhs views of the
                        # padded tile (xb-1 = left, xb+1 = right); no u
                        l2 = w2t[0:K, 0:K]
                        for j in (0, 1):
                            win = pt[0:K, j * 512:(j + 1) * 512]
                            nc.tensor.matmul(
                                win, l2,
                                xt[0:K, xb + j * 512:xb + j * 512 + 512],
                                start=True, stop=False,
                                skip_group_check=True)
                            nc.tensor.matmul(
                                win, l1,
                                xt[0:K, xb - 1 + j * 512:
                                   xb - 1 + j * 512 + 512],
                                start=False, stop=False,
                                skip_group_check=True)
                            nc.tensor.matmul(
                                win, l1,
                                xt[0:K, xb + 1 + j * 512:
                                   xb + 1 + j * 512 + 512],
                                start=False, stop=True,
                                skip_group_check=True)
                        return do_copy("a" if mode == "x" else "v",
                                       ot[0:K, ob:ob + 1024],
                                       pt[0:K, 0:1024])
                    else:  # two-pass
                        l2 = w2t[0:K, 0:K]
                        for j in (0, 1):
                            win = pt[0:K, j * 512:(j + 1) * 512]
                            nc.tensor.matmul(
                                win, l1,
                                ut[0:K, ub + j * 512:ub + j * 512 + 512],
                                start=True, stop=False,
                                skip_group_check=True)
                            nc.tensor.matmul(
                                win, l2,
                                xt[0:K, xb + j * 512:xb + j * 512 + 512],
                                start=False, stop=True,
                                skip_group_check=True)
                        return do_copy(mode, ot[0:K, ob:ob + 1024],
                                       pt[0:K, 0:1024])

                def do_super(s, g):
                    xs_ = x[s]
                    ys_ = y[s]
                    if True:
                        first = nbk * g
                        r0 = RPB * first
                        in_view = mkAP(
                            tensor=xs_.tensor,
                            offset=xs_.offset + r0 * WP,
                            ap=[[WP, 128], [RPB * WP, nbk], [1, WP]])
                        xt = xp.tile([128, nbk * WP], fp16, tag="xt")
                        nc.sync.dma_start(
                            xt[:].rearrange("p (n w) -> p n w", n=nbk),
                            in_view)
                        ot = op.tile([128, nbk * W], fp16, tag="ot")
                        if variant == "dma":
                            nc.vector.tensor_copy(ot[0:1, 0:2], xt[0:1, 0:2])
                            out_view = mkAP(
                                tensor=ys_.tensor,
                                offset=ys_.offset + r0 * W,
                                ap=[[W, 126], [RPB * W, nbk], [1, W]])
                            eng_out.dma_start(
                                out_view,
                                ot[1:127, :].rearrange(
                                    "p (n w) -> p n w", n=nbk))
                            return
                        needs_u = bool(set(plan) & set("aAvBV"))
                        ut = None
                        if needs_u:
                            ut = up.tile([128, nbk * WP], fp16, tag="ut")
                            tte = nc.vector if tt_plan[
                                (s * 2 + g) % len(tt_plan)] == "v" \
                                else nc.gpsimd
                            if tt_gran == "super":
                                tte.tensor_tensor(
                                    ut[:, 0:nbk * WP - 2],
                                    xt[:, 0:nbk * WP - 2],
                                    xt[:, 2:nbk * WP], op=add)

                        def blk_tt(b):
                            # u for this block only (tt_gran == "vonly")
                            cb = b * WP
                            nc.vector.tensor_tensor(
                                ut[:, cb:cb + 1024], xt[:, cb:cb + 1024],
                                xt[:, cb + 2:cb + 1026], op=add)
                        for b in range(nbk):
                            if tt_gran == "pair" and b % 2 == 0:
                                lo = b * WP
                                hi = min((b + 2) * WP, nbk * WP)
                                nc.vector.tensor_tensor(
                                    ut[:, lo:hi - 2], xt[:, lo:hi - 2],
                                    xt[:, lo + 2:hi], op=add)
                            pt = pp.tile([128, 1024], f32, tag="pt")
                            if variant == "full":
                                mode = next_mode()
                                if tt_gran == "vonly" and mode in "aAvBV":
                                    blk_tt(b)
                                ceng = do_block(mode, 128, pt, ut, xt,
                                                b * WP, b * WP + 1, ot,
                                                b * W)
                                if out_gran == "block":
                                    blk_view = mkAP(
                                        tensor=ys_.tensor,
                                        offset=ys_.offset +
                                        (r0 + b * RPB) * W,
                                        ap=[[W, 126], [1, W]])
                                    deng = ceng if ceng is not nc.vector \
                                        else getattr(nc, v_out_eng)
                                    deng.dma_start(
                                        blk_view,
                                        ot[1:127, b * W:(b + 1) * W])
                            else:
                                for j in (0, 1):
                                    nc.tensor.matmul(
                                        pt[:, j * 512:(j + 1) * 512],
                                        w1t[:],
                                        ut[:, b * WP + j * 512:
                                           b * WP + j * 512 + 512],
                                        start=True, stop=True,
                                        skip_group_check=True)
                                nc.vector.tensor_copy(
                                    ot[0:1, b * W:b * W + 2], pt[0:1, 0:2])
                        if variant != "full" or out_gran == "super":
                            out_view = mkAP(
                                tensor=ys_.tensor,
                                offset=ys_.offset + r0 * W,
                                ap=[[W, 126], [RPB * W, nbk], [1, W]])
                            eng_out.dma_start(
                                out_view,
                                ot[1:127, :].rearrange("p (n w) -> p n w",
                                                       n=nbk))

                def do_tail(s):
                    # 18-row tail block: x_p rows 1008..1025 -> out rows
                    # 1008..1023 (psum rows 1..16)
                    xtc = xp.tile([18, WP], fp16, tag="xtc")
                    nc.sync.dma_start(xtc[:], x[s, TAIL_X0:TAIL_X0 + 18, :])
                    otc = op.tile([18, W], fp16, tag="otc")
                    if variant == "dma":
                        nc.vector.tensor_copy(otc[0:1, 0:2], xtc[0:1, 0:2])
                        eng_out.dma_start(y[s, TAIL_X0:TAIL_X0 + 16, :],
                                          otc[1:17, :])
                        return
                    utc = None
                    if set(plan) & set("aAvBV"):
                        utc = up.tile([18, WP], fp16, tag="utc")
                        getattr(nc, tail_tt_eng).tensor_tensor(
                            utc[:, 0:WP - 2], xtc[:, 0:WP - 2], xtc[:, 2:WP],
                            op=add)
                    ptc = pp.tile([128, 1024], f32, tag="pt")
                    if variant == "full":
                        ceng = do_block(next_mode(), 18, ptc, utc, xtc, 0, 1,
                                        otc, 0)
                        if out_gran == "block":
                            deng = ceng if ceng is not nc.vector \
                                else getattr(nc, v_out_eng)
                            deng.dma_start(y[s, TAIL_X0:TAIL_X0 + 16, :],
                                           otc[1:17, :])
                            return
                    else:
                        for j in (0, 1):
                            nc.tensor.matmul(
                                ptc[0:18, j * 512:(j + 1) * 512],
                                w1t[0:18, 0:18],
                                utc[:, j * 512:j * 512 + 512],
                                start=True, stop=True,
                                skip_group_check=True)
                        nc.vector.tensor_copy(otc[0:1, 0:2], ptc[0:1, 0:2])
                    eng_out.dma_start(y[s, TAIL_X0:TAIL_X0 + 16, :],
                                      otc[1:17, :])

                nsup = NBLK // nbk
                if tails_last:
                    for s in range(S_PER_CORE):
                        for g in range(nsup):
                            do_super(s, g)
                    for s in range(S_PER_CORE):
                        do_tail(s)
                else:
                    for s in range(S_PER_CORE):
                        for g in range(nsup):
                            do_super(s, g)
                        do_tail(s)

            if loop_r == 1:
                body()
            else:
                with tc.For_i(0, loop_r, 1):
                    body()

    nc.compile()
    nc.m = get_hw_module(nc.m)
    return nc


def _get_program():
    if "main" not in _PROGRAMS:
        _PROGRAMS["main"] = _build_program()
    return _PROGRAMS["main"]


def _make_weights(scale):
    c1 = scale / 12.0
    c2 = 1.0 - 4.0 * scale / 3.0
    tri = np.zeros((128, 128), dtype=np.float64)
    idx = np.arange(128)
    tri[idx, idx] = 2.0
    tri[idx[:-1], idx[:-1] + 1] = 1.0
    tri[idx[:-1] + 1, idx[:-1]] = 1.0
    w1 = (c1 * tri).astype(np.float16)
    w2 = (2.0 * c1 * tri + c2 * np.eye(128)).astype(np.float16)
    c2v = np.full((128, 1), c2, dtype=np.float32)
    return w1, w2, c2v


def kernel(state, diffusion_coefficient, dt):
    from concourse.bass_utils import run_bass_kernel_spmd

    state = np.asarray(state)
    in_dtype = state.dtype
    xs = np.ascontiguousarray(state, dtype=np.float16).reshape(32, H, W)
    # periodic wrap pre-padded on host: [32, 1026, 1026]
    xpad = np.pad(xs, ((0, 0), (1, 1), (1, 1)), mode="wrap")

    scale = float(np.asarray(diffusion_coefficient, dtype=np.float64)) * \
        float(np.asarray(dt, dtype=np.float64))
    w1, w2, c2v = _make_weights(scale)

    nc = _get_program()
    in_maps = [
        {"x": xpad[k * S_PER_CORE:(k + 1) * S_PER_CORE], "w1": w1, "w2": w2,
         "c2v": c2v}
        for k in range(N_CORES)
    ]
    res = run_bass_kernel_spmd(nc, in_maps, core_ids=list(range(N_CORES)))
    out = np.concatenate([res.results[k]["y"] for k in range(N_CORES)], axis=0)
    return out.reshape(4, 8, H, W).astype(in_dtype, copy=False)
